# revision 6
# baseline (speedup 1.0000x reference)
"""Multi-head self-attention TRN2 Bass kernel.

Reference computation (per batch n):
  q = wq @ x; k = wk @ x; v = wv @ x            (1x1 conv == channel matmul)
  per (t, h): score = (q_th^T k_th) * sqrt(D); attn = softmax(score)
  o = attn @ v;  out = wp @ o + bp

Sharding: data-parallel over N=8 across the 8 NeuronCores (one batch each).

Precision: single-pass fp16 everywhere (gate is 2e-2 rel; softmax logits are
sharply peaked so fp16 q/k errors barely move the output). sqrt(D) folded
into wq host-side; x pre-cast to fp16 host-side; output DMA'd as fp16 and
cast back to fp32 on host.

Engine balance per (tl, bb) softmax step:
  PE: 4 score MMs f16 + 4 attn@v MMs f16
  DVE: negmax reduce (PSUM), attn = exp / rowsum (divide, fused normalize)
  ACT: 4x exp with bias=-max and accum_out=rowsum (no DVE sum pass)
  gpsimd: q/k/o PSUM->SBUF copies
  DMA: attn transpose (16x128 xbar tiles)
"""
import numpy as np
from contextlib import ExitStack

import concourse.bass as bass
import concourse.tile as tile
from concourse import bacc, mybir
from concourse.bass_utils import run_bass_kernel_spmd

N, C, T, V = 8, 256, 128, 128
OUT, H, D = 512, 8, 64
TV = T * V
TC = 8              # t-values per pipeline chunk
NCHUNK = T // TC    # 16
F32 = mybir.dt.float32
F16 = mybir.dt.float16

_CACHE = {}


def _build(nchunk=NCHUNK, debug=False):
    nc = bacc.Bacc("TRN2", target_bir_lowering=False, debug=debug)
    x_d = nc.dram_tensor("x", (C, TV), F16, kind="ExternalInput")
    wq_d = nc.dram_tensor("wqt", (C, OUT), F16, kind="ExternalInput")
    wk_d = nc.dram_tensor("wkt", (C, OUT), F16, kind="ExternalInput")
    wvt_d = nc.dram_tensor("wvt", (C, OUT), F16, kind="ExternalInput")
    wpt_d = nc.dram_tensor("wpt", (OUT, OUT), F16, kind="ExternalInput")
    bp_d = nc.dram_tensor("bpr", (128, 4), F32, kind="ExternalInput")
    out_d = nc.dram_tensor("out", (OUT, TV), F16, kind="ExternalOutput")

    with ExitStack() as ctx:
        tc = ctx.enter_context(tile.TileContext(nc))
        singles = ctx.enter_context(tc.tile_pool(name="singles", bufs=1))
        xpool = ctx.enter_context(tc.tile_pool(name="xp", bufs=2))
        qkpool = ctx.enter_context(tc.tile_pool(name="qk", bufs=2))
        vpool = ctx.enter_context(tc.tile_pool(name="vp", bufs=2))
        atpool = ctx.enter_context(tc.tile_pool(name="at", bufs=8))
        stats = ctx.enter_context(tc.tile_pool(name="st", bufs=8))
        opool = ctx.enter_context(tc.tile_pool(name="op", bufs=2))
        outpool = ctx.enter_context(tc.tile_pool(name="outp", bufs=2))
        projps = ctx.enter_context(tc.tile_pool(name="pps", bufs=2, space="PSUM"))
        scoreps = ctx.enter_context(tc.tile_pool(name="sps", bufs=3, space="PSUM"))
        ops_ps = ctx.enter_context(tc.tile_pool(name="ops", bufs=2, space="PSUM"))

        FC = TC * V  # free size per chunk (1024)

        def load_x(cc):
            tv0 = cc * FC
            x_t = xpool.tile([128, 2, FC], F16, tag="x", name="x_t")
            nc.sync.dma_start(
                x_t[:],
                x_d[:].rearrange("(a p) f -> p a f", a=2)[:, :, tv0:tv0 + FC])
            return x_t

        xs = {0: load_x(0)}

        # --- weights to SBUF (once) ---
        wq_sb = singles.tile([128, 2, OUT], F16, tag="wq")
        wk_sb = singles.tile([128, 2, OUT], F16, tag="wk")
        nc.sync.dma_start(wq_sb[:], wq_d[:].rearrange("(a p) f -> p a f", a=2))
        nc.sync.dma_start(wk_sb[:], wk_d[:].rearrange("(a p) f -> p a f", a=2))
        wvt_sb = singles.tile([128, 2, OUT], F16, tag="wvt")
        nc.sync.dma_start(wvt_sb[:], wvt_d[:].rearrange("(a p) f -> p a f", a=2))
        wpt_sb = singles.tile([128, 4, OUT], F16, tag="wpt")
        nc.sync.dma_start(wpt_sb[:], wpt_d[:].rearrange("(a p) f -> p a f", a=4))
        bp_sb = singles.tile([128, 4], F32, tag="bp")
        nc.sync.dma_start(bp_sb[:], bp_d[:])

        for cc in range(nchunk):
            tv0 = cc * FC
            if cc + 1 < nchunk:
                xs[cc + 1] = load_x(cc + 1)
            x_t = xs.pop(cc)

            # --- q, k projections (single-pass fp16), output f16 ---
            q_sb = qkpool.tile([128, 4, FC], F16, tag="q")
            k_sb = qkpool.tile([128, 4, FC], F16, tag="k")
            for dst, w_sb in ((q_sb, wq_sb), (k_sb, wk_sb)):
                for ot in range(4):
                    for nb in range(2):
                        ps = projps.tile([128, 512], F32, tag="pps", name="pps")
                        for ct in range(2):
                            nc.tensor.matmul(
                                ps[:],
                                w_sb[:, ct, ot * 128:(ot + 1) * 128],
                                x_t[:, ct, nb * 512:(nb + 1) * 512],
                                start=(ct == 0), stop=(ct == 1))
                        nc.vector.tensor_copy(
                            dst[:, ot, nb * 512:(nb + 1) * 512], ps[:])

            # --- v projection, transposed layout: vT[t] = [tokens(128), OUT] ---
            vT = vpool.tile([128, TC, OUT], F16, tag="vT")
            for tl in range(TC):
                ps = projps.tile([128, 512], F32, tag="pps", name="pps")
                for ct in range(2):
                    nc.tensor.matmul(ps[:],
                                     x_t[:, ct, tl * 128:(tl + 1) * 128],
                                     wvt_sb[:, ct, :],
                                     start=(ct == 0), stop=(ct == 1))
                nc.scalar.copy(vT[:, tl, :], ps[:])

            # --- attention ---
            for tl in range(TC):
                o_ps = ops_ps.tile([128, 4, 128], F32, tag="ops", name="o_ps")
                for bb in range(2):  # 4-instance batches over heads
                    sps = scoreps.tile([128, 4, 128], F32, tag="sps")
                    negmax = stats.tile([128, 4], F32, tag="negmax")
                    rowsum = stats.tile([128, 4], F32, tag="rowsum")
                    for s in range(4):
                        h = 2 * s + bb
                        ot, po = h // 2, (h % 2) * 64
                        nc.tensor.matmul(
                            sps[:, s, :],
                            q_sb[po:po + 64, ot, tl * 128:(tl + 1) * 128],
                            k_sb[po:po + 64, ot, tl * 128:(tl + 1) * 128],
                            start=True, stop=True)
                    nc.vector.tensor_reduce(negmax[:], sps[:],
                                            axis=mybir.AxisListType.X,
                                            op=mybir.AluOpType.max, negate=True)
                    exp_t = atpool.tile([128, 4, 128], F16, tag="exp")
                    for s in range(4):
                        nc.scalar.activation(exp_t[:, s, :], sps[:, s, :],
                                             mybir.ActivationFunctionType.Exp,
                                             bias=negmax[:, s:s + 1],
                                             accum_out=rowsum[:, s:s + 1])
                    recip = stats.tile([128, 4], F32, tag="recip")
                    nc.vector.reciprocal(recip[:], rowsum[:])
                    attn_t = atpool.tile([128, 4, 128], F16, tag="attn")
                    nc.gpsimd.tensor_tensor(
                        out=attn_t[:], in0=exp_t[:],
                        in1=recip[:].unsqueeze(2).broadcast_to([128, 4, 128]),
                        op=mybir.AluOpType.mult)
                    attnT = atpool.tile([128, 4, 128], F16, tag="attnT")
                    nc.sync.dma_start_transpose(attnT[:], attn_t[:])
                    for s in range(4):
                        h = 2 * s + bb
                        ot, po = h // 2, (h % 2) * 64
                        nc.tensor.matmul(
                            o_ps[po:po + 64, ot, :],
                            vT[:, tl, h * 64:(h + 1) * 64],
                            attnT[:, s, :],
                            start=True, stop=True)
                # stage o (natural layout) for the output projection
                g, tg = tl // 4, tl % 4
                if tg == 0:
                    o_g = opool.tile([128, 4, 4, 128], F16, tag="og")
                nc.vector.tensor_copy(o_g[:, :, tg, :], o_ps[:])
                if tg == 3:
                    outsb = outpool.tile([128, 4, 512], F16, tag="out", name="outsb")
                    for mt in range(4):
                        ps = projps.tile([128, 512], F32, tag="pps", name="pps")
                        for kt in range(4):
                            nc.tensor.matmul(ps[:],
                                             wpt_sb[:, kt, mt * 128:(mt + 1) * 128],
                                             o_g[:, kt, :, :],
                                             start=(kt == 0), stop=(kt == 3))
                        nc.scalar.add(outsb[:, mt, :], ps[:], bp_sb[:, mt:mt + 1])
                    nc.sync.dma_start(
                        out_d[:].rearrange("(a p) f -> p a f", a=4)
                        [:, :, tv0 + g * 512:tv0 + (g + 1) * 512],
                        outsb[:])

    nc.compile()
    return nc


def _prep_weights(wq, wk, wv, wp, bp):
    wqt = (wq.astype(np.float32) * 8.0).T.astype(np.float16)  # fold sqrt(D)=8
    wkt = wk.T.astype(np.float16)
    wvt = wv.T.astype(np.float16)
    wpt = wp.T.astype(np.float16)
    bpr = np.ascontiguousarray(bp.astype(np.float32).reshape(4, 128).T)
    return dict(wqt=wqt, wkt=wkt, wvt=wvt, wpt=wpt, bpr=bpr)


def kernel(x, wq, wk, wv, wp, bp):
    x = np.asarray(x, dtype=np.float16)
    w = _prep_weights(np.asarray(wq), np.asarray(wk), np.asarray(wv),
                      np.asarray(wp), np.asarray(bp))
    if "nc" not in _CACHE:
        _CACHE["nc"] = _build()
    nc = _CACHE["nc"]
    in_maps = []
    for n in range(N):
        m = dict(w)
        m["x"] = np.ascontiguousarray(x[n].reshape(C, TV))
        in_maps.append(m)
    res = run_bass_kernel_spmd(nc, in_maps, core_ids=list(range(N)))
    out = np.stack([r["out"].astype(np.float32).reshape(OUT, T, V)
                    for r in res.results])
    return out
